# revision 41
# baseline (speedup 1.0000x reference)
"""Bass/Tile kernel for nn_MicrotubuleAttention on 8 Trainium2 NeuronCores.

Math: the reference adds (1 - gtp) * NEG (NEG = -1e9) to every causal
off-diagonal score. With gamma clipped to >= 1e-4, the smallest penalty is
-1e9 * (1 - exp(-1e-4)) ~= -1e5, so after float32 softmax every off-diagonal
weight underflows to exactly 0 and attention is exactly the identity. Hence:

    out = repeat_gqa(x @ Wv) @ Wo = (x @ Wv) @ Wo_folded

where Wo_folded[64*g + d, :] = sum_r Wo[(4g+r)*64 + d, :] sums the 4
query-head row blocks that share KV head g. Q/K/RoPE/polarity/gamma provably
do not affect the f32 output.

Final design (48.1us baseline -> 30.0-33.3us measured across runs,
best 29.99us; run-to-run variance ~+-1.5us with identical code;
lineage 38.4 -> 36.6 -> 32.6 -> 31.8 -> 30.0):
- All inputs uploaded pre-rounded to bf16 in compute-ready layouts (pure
  layout transforms + the same bf16 rounding the baseline applied on
  device). Output stored bf16, upcast on host. Per-core HBM 9MB -> 4.5MB.
- x uploads PRE-TRANSPOSED and INTERLEAVED with Wv per contraction chunk:
  xw[p, kk, 0:512] = x[m, 128kk+p], xw[p, kk, 512:768] = Wv[128kk+p, :].
  Six chunk DMAs (1,1,2,2,1,1 kk; small leading chunks for an early PE
  start) alternate across both HWDGE rings with per-partition-contiguous
  runs; stage 1's 16 matmuls chase the stream from ~11us (v3 waited for
  the full 1.5MB before the first matmul).
- Full GQA fold of Wo -> [256, 1024] on DVE (12 bf16 [128,512] adds in
  wo-chunk arrival order): both matmul stages contract over 256 (16
  matmuls each) vs the baseline's 512-with-duplication (32+32). Wo
  uploads in fold-ready layout [p=(64gl+d), jc, nh, r, n'], loaded as 4
  512KB chunks (4KB runs), jc0 halves first, queued last on each ring -
  they are the latest-needed bytes (stage-2 rhs).
- Loads are chip-HBM-bound (8 cores x 3.5MB =~ 28MB at ~2.1-2.7TB/s,
  ~260-340GB/s/core): the ring schedule packs bytes in need-order so PE
  runs ~95% busy from first chunk to last stage-2 matmul.
- Stage 2: jc0 pass nh-outer (chases wof halves), jc1 stop pass mi-outer
  with per-row-block PSUM copies (ACT+DVE split) and stores; the last
  row block's store rides the otherwise-idle scalar ring.
- Bacc(enable_partition_id=False): drops the per-engine partition-id
  TENSOR_LOAD round from the preamble (~0.3-0.5us; the kernel never
  reads it).
- Measured dead ends (do not retry without new evidence): ncfw
  collectives (cold AllGather 50-80us, ~12.5us warm; sequential core
  dispatch means ~1.2ms/core start skew without has_collectives);
  remote_dma_broadcast mesh-AllGather of folded Wo slices (works and is
  fast with nc.has_collectives=True forcing synchronized launch, but
  intermittently delivered garbage on 2/8 cores - stale-semaphore
  hazard); >12-matmul HAM warmup bursts (trigger chip-wide power
  throttling: 0.43us matmuls -> 0.76us, DVE adds 3x slower); GpSimd
  tensor_add folds (1.2-1.6us vs DVE 0.42us); per-kk 192KB load chunks
  with 1.5KB runs (drops rings to ~110GB/s); xbar DMA-transpose of x
  (256B descriptors flood the SDMA fabric); interleaving PSUM copies
  between stage-2 stop matmuls (scheduler serializes, +4us); tiny
  queue-warming DMAs ahead of chunk0 (just delay it); half-row stores
  (more DMAs, 1KB runs, no gain); jc1-wo on a SWDGE third stream
  (steals SDMA slots from the rings, +3us); splitting the jc1 wo chunks
  into 256KB r-halves to pipeline their completion sems (2KB runs +
  extra DMAs cost more than the ~1us sem-latency it hides); filler
  warmup matmuls inside stage-1's chunk-wait stall (statistically
  neutral); splitting the vT cast per row block (ACTIVATE has ~0.7us
  FIXED cost regardless of width - splitting multiplies it, +2us).
  Note: every load/store DMA's completion sem fires ~1.5-2us after its
  last data packet (AXI receipt round trip) - chunk-consumer stalls and
  the post-store drain are sem-latency, not bandwidth. The stage-1-end
  chain and the wo-fold chain converge within ~0.1us at the stop pass:
  the design is balanced, so single-chain micro-gains move nothing.

Sharding: data parallel over rows. B*T = 4096 rows split 8 ways -> 512
rows per core; Wv/Wo broadcast.
"""

import os
import sys

import numpy as np
import ml_dtypes

for _p in ("/opt/trn_rl_repo", "/opt/pypackages"):
    if os.path.isdir(_p) and _p not in sys.path:
        sys.path.append(_p)

B, T, D_MODEL = 2, 2048, 1024
H_Q, H_KV, D_HEAD = 16, 4, 64
N_CORES = 8
M_TOTAL = B * T              # 4096 rows
M_CORE = M_TOTAL // N_CORES  # 512 rows per core
P = 128
KK = D_MODEL // P            # 8 contraction chunks of 128
MC = M_CORE // P             # 4 row chunks of 128
NKV = H_KV * D_HEAD          # 256
XW = M_CORE + NKV            # 768: x chunk (512) || wv chunk (256)
BF = ml_dtypes.bfloat16

TRACE = False          # test.py flips this to profile
TRACE_CORES = None
LAST_RESULTS = None    # BassKernelResults of the most recent run

_nc_cache = None


def _build_bass():
    import concourse.bass as bass
    import concourse.mybir as mybir
    import concourse.tile as tile
    from concourse import bacc
    from concourse.masks import make_identity
    from concourse.tile import add_dep_helper

    f32 = mybir.dt.float32
    bf16 = mybir.dt.bfloat16
    ts = bass.ts

    def dep(later, earlier, reason="order"):
        add_dep_helper(later.ins, earlier.ins, reason=reason)

    nc = bacc.Bacc(None, enable_partition_id=False)
    xw_d = nc.declare_dram_parameter("xw", [P, KK, XW], bf16, isOutput=False)
    wo_d = nc.declare_dram_parameter("wo", [P, 2, 2, 4, 512], bf16, isOutput=False)
    out_d = nc.declare_dram_parameter("out", [M_CORE, D_MODEL], bf16, isOutput=True)

    with tile.TileContext(nc) as tc:
        with (
            tc.tile_pool(name="const", bufs=1) as const,
            tc.tile_pool(name="tmp", bufs=2) as tmp,
            tc.tile_pool(name="o_pool", bufs=4) as o_pool,
            tc.tile_pool(name="psum", bufs=8, space="PSUM") as psum,
        ):
            ident_bf = const.tile([P, P], bf16)
            make_identity(nc, ident_bf)

            xw_sb = const.tile([P, KK, XW], bf16)      # [k_lo, kk, m||j]
            wo_sb = const.tile([P, 2, 2, 4, 512], bf16)  # [64gl+d, jc, nh, r, n']
            wof = const.tile([P, 2, D_MODEL], bf16)    # [64gl+d, jc, n] folded
            vT_sb = const.tile([P, 2, M_CORE], bf16)   # [j_lo, jc, m]

            # ---- loads: all (x||wv) chunks first (stage-1 stream; first
            # chunks small so PE starts early), then the four wo chunks
            # (4KB runs), jc0 halves before jc1 (stage-2 need order).
            rings = [nc.scalar, nc.sync]
            xw_chunks = [(0, 1), (1, 2), (2, 4), (4, 6), (6, 7), (7, 8)]
            for ci, (k0, k1) in enumerate(xw_chunks):
                rings[ci % 2].dma_start(
                    xw_sb[:, k0:k1, :], xw_d[:, k0:k1, :])
            for jc in range(2):
                for nh in range(2):
                    rings[nh].dma_start(
                        wo_sb[:, jc, nh, :, :],
                        wo_d[:, jc, nh, :, :],
                    )

            # ---- PE HAM warmup while loads land (modest: a big burst here
            # measurably TRIGGERS power throttling chip-wide)
            warm = psum.tile([P, P], f32, tag="ps")
            for _ in range(12):
                nc.tensor.matmul(warm[:], lhsT=ident_bf[:], rhs=ident_bf[:],
                                 start=True, stop=True)

            # ---- stage 1: vT[j, m] = sum_k Wv[k, j] x[m, k]; chases the
            # per-kk chunk arrivals; jc interleaved across two PSUM banks.
            ps1 = [psum.tile([P, M_CORE], f32, tag="ps", name=f"ps1_{jc}")
                   for jc in range(2)]
            for kk in range(KK):
                for jc in range(2):
                    nc.tensor.matmul(
                        ps1[jc][:],
                        lhsT=xw_sb[:, kk, M_CORE + 128 * jc : M_CORE + 128 * (jc + 1)],
                        rhs=xw_sb[:, kk, 0:M_CORE],
                        start=(kk == 0),
                        stop=(kk == KK - 1),
                    )

            nc.scalar.copy(vT_sb[:, 0, :], ps1[0][:])
            nc.scalar.copy(vT_sb[:, 1, :], ps1[1][:])

            # ---- GQA fold: wof[:, jc, 512nh+n'] = sum_r wo_sb[:, jc, nh, r, n'].
            # All on DVE (GpSimd adds measured 3x slower and drag a slow
            # window with them), emitted in wo-chunk arrival order.
            for jc in range(2):
                for nh in range(2):
                    eng = nc.vector
                    t01 = tmp.tile([P, 512], bf16, tag=f"t01_{nh}",
                                   name=f"t01_{jc}_{nh}")
                    t23 = tmp.tile([P, 512], bf16, tag=f"t23_{nh}",
                                   name=f"t23_{jc}_{nh}")
                    eng.tensor_add(
                        t01[:], wo_sb[:, jc, nh, 0, :], wo_sb[:, jc, nh, 1, :])
                    eng.tensor_add(
                        t23[:], wo_sb[:, jc, nh, 2, :], wo_sb[:, jc, nh, 3, :])
                    eng.tensor_add(
                        wof[:, jc, ts(nh, 512)], t01[:], t23[:])

            # ---- stage 2: out[m, n] = sum_j v[m, j] Wo_f[j, n]; 8 live
            # PSUM tiles, accumulate over jc. jc0 pass ordered nh-outer
            # (chases the wof halves); jc1 (stop) pass mi-outer so each
            # row block's copies + store fire as soon as it stops.
            ps2 = {}
            for mi in range(MC):
                for nh2 in range(2):
                    ps2[(mi, nh2)] = psum.tile(
                        [P, 512], f32, tag="ps", name=f"ps2_{mi}_{nh2}")
            for nh2 in range(2):
                for mi in range(MC):
                    nc.tensor.matmul(
                        ps2[(mi, nh2)][:],
                        lhsT=vT_sb[:, 0, ts(mi, P)],
                        rhs=wof[:, 0, ts(nh2, 512)],
                        start=True,
                        stop=False,
                    )
            for mi in range(MC):
                for nh2 in range(2):
                    nc.tensor.matmul(
                        ps2[(mi, nh2)][:],
                        lhsT=vT_sb[:, 1, ts(mi, P)],
                        rhs=wof[:, 1, ts(nh2, 512)],
                        start=False,
                        stop=True,
                    )
                o_sb = o_pool.tile([P, D_MODEL], bf16, tag="o_sb",
                                   name=f"o_{mi}")
                nc.scalar.copy(o_sb[:, 0:512], ps2[(mi, 0)][:])
                nc.vector.tensor_copy(o_sb[:, 512:1024], ps2[(mi, 1)][:])
                # last store rides the otherwise-idle scalar ring (earlier
                # mi's would block ACT's remaining copies via issue-wait)
                st_eng = nc.scalar if mi == MC - 1 else nc.sync
                st_eng.dma_start(out_d[ts(mi, P), :], o_sb[:])

    nc.finalize()
    return nc


def _get_nc():
    global _nc_cache
    if _nc_cache is None:
        _nc_cache = _build_bass()
    return _nc_cache


def _prep_shared(inputs):
    """Host-side layout transforms + bf16 rounding (shared across cores)."""
    # wv2[p, kk, j] = Wv[128*kk + p, j]
    wv = (
        np.asarray(inputs["Wv"], dtype=np.float32)
        .reshape(KK, P, NKV).transpose(1, 0, 2)
    ).astype(BF)
    # woA[64*gl + d, jc, nh, r, n'] = Wo[256*(2*jc + gl) + 64*r + d, 512*nh + n']
    wo = np.asarray(inputs["Wo"], dtype=np.float32)
    woA = np.ascontiguousarray(
        wo.reshape(2, 2, 4, 64, 2, 512)       # (jc, gl, r, d, nh, n')
        .transpose(1, 3, 0, 4, 2, 5)          # (gl, d, jc, nh, r, n')
        .reshape(P, 2, 2, 4, 512)
    ).astype(BF)
    return wv, woA


def kernel(**inputs) -> np.ndarray:
    global LAST_RESULTS
    from concourse.bass_utils import run_bass_kernel_spmd

    x = np.asarray(inputs["x"], dtype=np.float32).reshape(M_TOTAL, D_MODEL)
    xt = x.reshape(M_TOTAL, KK, P).astype(BF)
    wvb, woA = _prep_shared(inputs)

    in_maps = []
    for i in range(N_CORES):
        # xw[p, kk, :] = [ x[m, 128kk+p] for m in core rows | Wv[128kk+p, :] ]
        xw = np.empty((P, KK, XW), dtype=BF)
        xw[:, :, :M_CORE] = xt[i * M_CORE : (i + 1) * M_CORE].transpose(2, 1, 0)
        xw[:, :, M_CORE:] = wvb
        in_maps.append({"xw": np.ascontiguousarray(xw), "wo": woA})

    nc = _get_nc()
    res = run_bass_kernel_spmd(
        nc,
        in_maps,
        list(range(N_CORES)),
        trace=TRACE,
        trace_cores=TRACE_CORES,
    )
    LAST_RESULTS = res
    out = np.concatenate(
        [np.asarray(r["out"]) for r in res.results], axis=0
    ).astype(np.float32)
    return out.reshape(B, T, D_MODEL)


# revision 42
# speedup vs baseline: 1.1101x; 1.1101x over previous
"""Bass/Tile kernel for nn_MicrotubuleAttention on 8 Trainium2 NeuronCores.

Math: the reference adds (1 - gtp) * NEG (NEG = -1e9) to every causal
off-diagonal score. With gamma clipped to >= 1e-4, the smallest penalty is
-1e9 * (1 - exp(-1e-4)) ~= -1e5, so after float32 softmax every off-diagonal
weight underflows to exactly 0 and attention is exactly the identity. Hence:

    out = repeat_gqa(x @ Wv) @ Wo = (x @ Wv) @ Wo_folded

where Wo_folded[64*g + d, :] = sum_r Wo[(4g+r)*64 + d, :] sums the 4
query-head row blocks that share KV head g. Q/K/RoPE/polarity/gamma provably
do not affect the f32 output.

Final design (48.1us baseline -> 30.0-33.3us measured across runs,
best 29.99us; run-to-run variance ~+-1.5us with identical code;
lineage 38.4 -> 36.6 -> 32.6 -> 31.8 -> 30.0):
- All inputs uploaded pre-rounded to bf16 in compute-ready layouts (pure
  layout transforms + the same bf16 rounding the baseline applied on
  device). Output stored bf16, upcast on host. Per-core HBM 9MB -> 4.5MB.
- x uploads PRE-TRANSPOSED and INTERLEAVED with Wv per contraction chunk:
  xw[p, kk, 0:512] = x[m, 128kk+p], xw[p, kk, 512:768] = Wv[128kk+p, :].
  Six chunk DMAs (1,1,2,2,1,1 kk; small leading chunks for an early PE
  start) alternate across both HWDGE rings with per-partition-contiguous
  runs; stage 1's 16 matmuls chase the stream from ~11us (v3 waited for
  the full 1.5MB before the first matmul).
- Full GQA fold of Wo -> [256, 1024] on DVE (12 bf16 [128,512] adds in
  wo-chunk arrival order): both matmul stages contract over 256 (16
  matmuls each) vs the baseline's 512-with-duplication (32+32). Wo
  uploads in fold-ready layout [p=(64gl+d), jc, nh, r, n'], loaded as 4
  512KB chunks (4KB runs), jc0 halves first, queued last on each ring -
  they are the latest-needed bytes (stage-2 rhs).
- Loads are chip-HBM-bound (8 cores x 3.5MB =~ 28MB at ~2.1-2.7TB/s,
  ~260-340GB/s/core): the ring schedule packs bytes in need-order so PE
  runs ~95% busy from first chunk to last stage-2 matmul.
- Stage 2: jc0 pass nh-outer (chases wof halves), jc1 stop pass mi-outer
  with per-row-block PSUM copies (ACT+DVE split) and stores; the last
  row block's store rides the otherwise-idle scalar ring.
- Bacc(enable_partition_id=False): drops the per-engine partition-id
  TENSOR_LOAD round from the preamble (~0.3-0.5us; the kernel never
  reads it).
- Measured dead ends (do not retry without new evidence): ncfw
  collectives (cold AllGather 50-80us, ~12.5us warm; sequential core
  dispatch means ~1.2ms/core start skew without has_collectives);
  remote_dma_broadcast mesh-AllGather of folded Wo slices (works and is
  fast with nc.has_collectives=True forcing synchronized launch, but
  intermittently delivered garbage on 2/8 cores - stale-semaphore
  hazard); >12-matmul HAM warmup bursts (trigger chip-wide power
  throttling: 0.43us matmuls -> 0.76us, DVE adds 3x slower); GpSimd
  tensor_add folds (1.2-1.6us vs DVE 0.42us); per-kk 192KB load chunks
  with 1.5KB runs (drops rings to ~110GB/s); xbar DMA-transpose of x
  (256B descriptors flood the SDMA fabric); interleaving PSUM copies
  between stage-2 stop matmuls (scheduler serializes, +4us); tiny
  queue-warming DMAs ahead of chunk0 (just delay it); half-row stores
  (more DMAs, 1KB runs, no gain); jc1-wo on a SWDGE third stream
  (steals SDMA slots from the rings, +3us); splitting the jc1 wo chunks
  into 256KB r-halves to pipeline their completion sems (2KB runs +
  extra DMAs cost more than the ~1us sem-latency it hides); filler
  warmup matmuls inside stage-1's chunk-wait stall (statistically
  neutral); splitting the vT cast per row block (ACTIVATE has ~0.7us
  FIXED cost regardless of width - splitting multiplies it, +2us).
  Note: every load/store DMA's completion sem fires ~1.5-2us after its
  last data packet (AXI receipt round trip) - chunk-consumer stalls and
  the post-store drain are sem-latency, not bandwidth. The stage-1-end
  chain and the wo-fold chain converge within ~0.1us at the stop pass:
  the design is balanced, so single-chain micro-gains move nothing.

Sharding: data parallel over rows. B*T = 4096 rows split 8 ways -> 512
rows per core; Wv/Wo broadcast.
"""

import os
import sys

import numpy as np
import ml_dtypes

for _p in ("/opt/trn_rl_repo", "/opt/pypackages"):
    if os.path.isdir(_p) and _p not in sys.path:
        sys.path.append(_p)

B, T, D_MODEL = 2, 2048, 1024
H_Q, H_KV, D_HEAD = 16, 4, 64
N_CORES = 8
M_TOTAL = B * T              # 4096 rows
M_CORE = M_TOTAL // N_CORES  # 512 rows per core
P = 128
KK = D_MODEL // P            # 8 contraction chunks of 128
MC = M_CORE // P             # 4 row chunks of 128
NKV = H_KV * D_HEAD          # 256
XW = M_CORE + NKV            # 768: x chunk (512) || wv chunk (256)
BF = ml_dtypes.bfloat16

TRACE = False          # test.py flips this to profile
TRACE_CORES = None
LAST_RESULTS = None    # BassKernelResults of the most recent run

_nc_cache = None


def _build_bass():
    import concourse.bass as bass
    import concourse.mybir as mybir
    import concourse.tile as tile
    from concourse import bacc
    from concourse.masks import make_identity
    from concourse.tile import add_dep_helper

    f32 = mybir.dt.float32
    bf16 = mybir.dt.bfloat16
    ts = bass.ts

    def dep(later, earlier, reason="order"):
        add_dep_helper(later.ins, earlier.ins, reason=reason)

    nc = bacc.Bacc(None, enable_partition_id=False)
    xw_d = nc.declare_dram_parameter("xw", [P, KK, XW], bf16, isOutput=False)
    wo_d = nc.declare_dram_parameter("wo", [P, 2, 2, 4, 512], bf16, isOutput=False)
    out_d = nc.declare_dram_parameter("out", [M_CORE, D_MODEL], bf16, isOutput=True)

    with tile.TileContext(nc) as tc:
        with (
            tc.tile_pool(name="const", bufs=1) as const,
            tc.tile_pool(name="tmp", bufs=2) as tmp,
            tc.tile_pool(name="o_pool", bufs=4) as o_pool,
            tc.tile_pool(name="psum", bufs=8, space="PSUM") as psum,
        ):
            ident_bf = const.tile([P, P], bf16)
            make_identity(nc, ident_bf)

            xw_sb = const.tile([P, KK, XW], bf16)      # [k_lo, kk, m||j]
            wo_sb = const.tile([P, 2, 2, 4, 512], bf16)  # [64gl+d, jc, nh, r, n']
            wof = const.tile([P, 2, D_MODEL], bf16)    # [64gl+d, jc, n] folded
            vT_sb = const.tile([P, 2, M_CORE], bf16)   # [j_lo, jc, m]

            # ---- loads: all (x||wv) chunks first (stage-1 stream; first
            # chunks small so PE starts early), then the four wo chunks
            # (4KB runs), jc0 halves before jc1 (stage-2 need order).
            rings = [nc.scalar, nc.sync]
            xw_chunks = [(0, 1), (1, 2), (2, 4), (4, 6), (6, 7), (7, 8)]
            for ci, (k0, k1) in enumerate(xw_chunks):
                rings[ci % 2].dma_start(
                    xw_sb[:, k0:k1, :], xw_d[:, k0:k1, :])
            for jc in range(2):
                for nh in range(2):
                    rings[nh].dma_start(
                        wo_sb[:, jc, nh, :, :],
                        wo_d[:, jc, nh, :, :],
                    )

            # ---- PE HAM warmup while loads land (modest: a big burst here
            # measurably TRIGGERS power throttling chip-wide)
            warm = psum.tile([P, P], f32, tag="ps")
            for _ in range(12):
                nc.tensor.matmul(warm[:], lhsT=ident_bf[:], rhs=ident_bf[:],
                                 start=True, stop=True)

            # ---- stage 1: vT[j, m] = sum_k Wv[k, j] x[m, k]; chases the
            # per-kk chunk arrivals; jc interleaved across two PSUM banks.
            ps1 = [psum.tile([P, M_CORE], f32, tag="ps", name=f"ps1_{jc}")
                   for jc in range(2)]
            for kk in range(KK):
                for jc in range(2):
                    nc.tensor.matmul(
                        ps1[jc][:],
                        lhsT=xw_sb[:, kk, M_CORE + 128 * jc : M_CORE + 128 * (jc + 1)],
                        rhs=xw_sb[:, kk, 0:M_CORE],
                        start=(kk == 0),
                        stop=(kk == KK - 1),
                    )

            nc.scalar.copy(vT_sb[:, 0, :], ps1[0][:])
            nc.scalar.copy(vT_sb[:, 1, :], ps1[1][:])

            # ---- GQA fold: wof[:, jc, 512nh+n'] = sum_r wo_sb[:, jc, nh, r, n'].
            # All on DVE (GpSimd adds measured 3x slower and drag a slow
            # window with them), emitted in wo-chunk arrival order.
            for jc in range(2):
                for nh in range(2):
                    eng = nc.vector
                    t01 = tmp.tile([P, 512], bf16, tag=f"t01_{nh}",
                                   name=f"t01_{jc}_{nh}")
                    t23 = tmp.tile([P, 512], bf16, tag=f"t23_{nh}",
                                   name=f"t23_{jc}_{nh}")
                    eng.tensor_add(
                        t01[:], wo_sb[:, jc, nh, 0, :], wo_sb[:, jc, nh, 1, :])
                    eng.tensor_add(
                        t23[:], wo_sb[:, jc, nh, 2, :], wo_sb[:, jc, nh, 3, :])
                    eng.tensor_add(
                        wof[:, jc, ts(nh, 512)], t01[:], t23[:])

            # ---- stage 2: out[m, n] = sum_j v[m, j] Wo_f[j, n]; 8 live
            # PSUM tiles, accumulate over jc. jc0 pass ordered nh-outer
            # (chases the wof halves); jc1 (stop) pass mi-outer so each
            # row block's copies + store fire as soon as it stops.
            ps2 = {}
            for mi in range(MC):
                for nh2 in range(2):
                    ps2[(mi, nh2)] = psum.tile(
                        [P, 512], f32, tag="ps", name=f"ps2_{mi}_{nh2}")
            for nh2 in range(2):
                for mi in range(MC):
                    nc.tensor.matmul(
                        ps2[(mi, nh2)][:],
                        lhsT=vT_sb[:, 0, ts(mi, P)],
                        rhs=wof[:, 0, ts(nh2, 512)],
                        start=True,
                        stop=False,
                    )
            for mi in range(MC):
                for nh2 in range(2):
                    nc.tensor.matmul(
                        ps2[(mi, nh2)][:],
                        lhsT=vT_sb[:, 1, ts(mi, P)],
                        rhs=wof[:, 1, ts(nh2, 512)],
                        start=False,
                        stop=True,
                    )
                o_sb = o_pool.tile([P, D_MODEL], bf16, tag="o_sb",
                                   name=f"o_{mi}")
                nc.scalar.copy(o_sb[:, 0:512], ps2[(mi, 0)][:])
                nc.vector.tensor_copy(o_sb[:, 512:1024], ps2[(mi, 1)][:])
                if mi < MC - 1:
                    nc.sync.dma_start(out_d[ts(mi, P), :], o_sb[:])
                elif os.environ.get("KERNEL_AB") == "B":
                    # B: last store split across BOTH rings so the halves
                    # transfer in parallel (last byte ~0.4us earlier)
                    nc.scalar.dma_start(out_d[ts(mi, P), 0:512],
                                        o_sb[:, 0:512])
                    nc.sync.dma_start(out_d[ts(mi, P), 512:1024],
                                      o_sb[:, 512:1024])
                else:
                    # A: last store whole on the otherwise-idle scalar ring
                    # (earlier mi's would block ACT's copies via issue-wait)
                    nc.scalar.dma_start(out_d[ts(mi, P), :], o_sb[:])

    nc.finalize()
    return nc


def _get_nc():
    global _nc_cache
    if _nc_cache is None:
        _nc_cache = _build_bass()
    return _nc_cache


def _prep_shared(inputs):
    """Host-side layout transforms + bf16 rounding (shared across cores)."""
    # wv2[p, kk, j] = Wv[128*kk + p, j]
    wv = (
        np.asarray(inputs["Wv"], dtype=np.float32)
        .reshape(KK, P, NKV).transpose(1, 0, 2)
    ).astype(BF)
    # woA[64*gl + d, jc, nh, r, n'] = Wo[256*(2*jc + gl) + 64*r + d, 512*nh + n']
    wo = np.asarray(inputs["Wo"], dtype=np.float32)
    woA = np.ascontiguousarray(
        wo.reshape(2, 2, 4, 64, 2, 512)       # (jc, gl, r, d, nh, n')
        .transpose(1, 3, 0, 4, 2, 5)          # (gl, d, jc, nh, r, n')
        .reshape(P, 2, 2, 4, 512)
    ).astype(BF)
    return wv, woA


def kernel(**inputs) -> np.ndarray:
    global LAST_RESULTS
    from concourse.bass_utils import run_bass_kernel_spmd

    x = np.asarray(inputs["x"], dtype=np.float32).reshape(M_TOTAL, D_MODEL)
    xt = x.reshape(M_TOTAL, KK, P).astype(BF)
    wvb, woA = _prep_shared(inputs)

    in_maps = []
    for i in range(N_CORES):
        # xw[p, kk, :] = [ x[m, 128kk+p] for m in core rows | Wv[128kk+p, :] ]
        xw = np.empty((P, KK, XW), dtype=BF)
        xw[:, :, :M_CORE] = xt[i * M_CORE : (i + 1) * M_CORE].transpose(2, 1, 0)
        xw[:, :, M_CORE:] = wvb
        in_maps.append({"xw": np.ascontiguousarray(xw), "wo": woA})

    nc = _get_nc()
    res = run_bass_kernel_spmd(
        nc,
        in_maps,
        list(range(N_CORES)),
        trace=TRACE,
        trace_cores=TRACE_CORES,
    )
    LAST_RESULTS = res
    out = np.concatenate(
        [np.asarray(r["out"]) for r in res.results], axis=0
    ).astype(np.float32)
    return out.reshape(B, T, D_MODEL)
